# revision 3
# baseline (speedup 1.0000x reference)
"""Trainium2 Bass kernel for the MGRU cell (nn_MGRUCell_67070209295131).

Data-parallel over 8 NeuronCores: batch axis (4096) sharded into 8 x 512.
Each core runs the full cell on its batch shard; weights are replicated.

Per-core math (B=512 rows, I=H=O=512, K=16):
  hidden = hiddens[15]
  u  = sigmoid([x,h] @ Wu + bu)
  r  = sigmoid([x,h] @ Wr + br)
  d  = 0.5*sigmoid([x,h,m] @ Wm + bm)          (memory_gate)
  ht = tanh([x, r*h] @ Wh + bh)
  filt = -sum_k hiddens[k] * w[k],  w = flip(cumprod((i-d)/(i+1)))
  hn = filt + u*ht
  y  = hn @ Wo + bo
  out_hiddens = [hiddens[1:], hn]

The filter is evaluated as a Horner recurrence streamed forward over k:
  s = hiddens[0];  s <- hiddens[k] + (i-d)/(i+1)*s  (i = 16-k, k=1..15)
  filt = d*s
Each Horner step is two fused scalar_tensor_tensor DVE ops on [128, 2048]
tiles (batch%128 on partitions, (btile, h) on free).
"""

import sys

sys.path.insert(0, "/opt/trn_rl_repo")

from contextlib import ExitStack

import numpy as np

import concourse.bass as bass
import concourse.mybir as mybir
import concourse.tile as tile
from concourse import bacc
from concourse import bass_utils
from concourse.masks import make_identity

N_CORES = 8
B_FULL = 4096
BL = B_FULL // N_CORES  # 512 rows per core
I = H = O = 512
K = 16
P = 128
NT = BL // P  # 4 batch tiles per core

F32 = mybir.dt.float32
AF = mybir.ActivationFunctionType
OP = mybir.AluOpType


def _r3(ap2d):
    # [BL, 512] dram view -> [P, NT, 512] (partition = b%P, free = (btile, h))
    return ap2d.rearrange("(t p) h -> p t h", p=P)


def _build(zero_bias: bool):
    nc = bacc.Bacc("TRN2", target_bir_lowering=False, debug=False)

    sample = nc.dram_tensor("sample", [BL, I], F32, kind="ExternalInput").ap()
    hiddens = nc.dram_tensor("hiddens", [K, BL, H], F32, kind="ExternalInput").ap()
    mem_para = nc.dram_tensor("mem_para", [BL, H], F32, kind="ExternalInput").ap()
    Wu = nc.dram_tensor("Wu", [I + H, H], F32, kind="ExternalInput").ap()
    Wr = nc.dram_tensor("Wr", [I + H, H], F32, kind="ExternalInput").ap()
    Wm = nc.dram_tensor("Wm", [I + 2 * H, H], F32, kind="ExternalInput").ap()
    Wh = nc.dram_tensor("Wh", [I + H, H], F32, kind="ExternalInput").ap()
    Wo = nc.dram_tensor("Wo", [H, O], F32, kind="ExternalInput").ap()
    if not zero_bias:
        b_aps = [
            nc.dram_tensor(n, [H], F32, kind="ExternalInput").ap()
            for n in ("bu", "br", "bm", "bh", "bo")
        ]
    out_y = nc.dram_tensor("out_y", [BL, O], F32, kind="ExternalOutput").ap()
    out_h = nc.dram_tensor("out_h", [K, BL, H], F32, kind="ExternalOutput").ap()
    out_m = nc.dram_tensor("out_m", [BL, H], F32, kind="ExternalOutput").ap()

    with tile.TileContext(nc) as tc, ExitStack() as ctx:
        const = ctx.enter_context(tc.tile_pool(name="const", bufs=1))
        persist = ctx.enter_context(tc.tile_pool(name="persist", bufs=1))
        wpool = ctx.enter_context(tc.tile_pool(name="w", bufs=8))
        hpool = ctx.enter_context(tc.tile_pool(name="hstream", bufs=5))
        scratch = ctx.enter_context(tc.tile_pool(name="scratch", bufs=3))
        gpsum = ctx.enter_context(
            tc.tile_pool(name="gpsum", bufs=4, space=bass.MemorySpace.PSUM)
        )
        tpsum = ctx.enter_context(
            tc.tile_pool(name="tpsum", bufs=4, space=bass.MemorySpace.PSUM)
        )

        identity = const.tile([P, P], F32, tag="identity")
        make_identity(nc, identity[:])
        ones = const.tile([1, 512], F32, tag="ones")
        nc.vector.memset(ones[:], 1.0)
        if not zero_bias:
            bias_sb = const.tile([5, 512], F32, tag="bias")
            for j, b_ap in enumerate(b_aps):
                nc.sync.dma_start(bias_sb[j : j + 1, :], b_ap[None, :])

        # ---- batch-major loads ----
        sample_bm = persist.tile([P, NT, 512], F32, tag="sample_bm")
        nc.sync.dma_start(sample_bm[:], _r3(sample))
        hidden_bm = persist.tile([P, NT, 512], F32, tag="hidden_bm")
        nc.sync.dma_start(hidden_bm[:], _r3(hiddens[K - 1]))
        mp_bm = persist.tile([P, NT, 512], F32, tag="mp_bm")
        nc.sync.dma_start(mp_bm[:], _r3(mem_para))

        # ---- feature-major (transposed) activations ----
        sampleT = persist.tile([P, NT, 512], F32, tag="sampleT")
        hiddenT = persist.tile([P, NT, 512], F32, tag="hiddenT")
        mpT = persist.tile([P, NT, 512], F32, tag="mpT")
        rhT = persist.tile([P, NT, 512], F32, tag="rhT")
        hnT = persist.tile([P, NT, 512], F32, tag="hnT")

        tp_count = 0

        def transpose_blocks(dst, src_bm):
            nonlocal tp_count
            for fc in range(4):
                for bt in range(NT):
                    pst = tpsum.tile([P, P], F32, tag="tp")
                    nc.tensor.transpose(
                        pst[:], src_bm[:, bt, fc * P : (fc + 1) * P], identity[:]
                    )
                    dst_ap = dst[:, fc, bt * P : (bt + 1) * P]
                    if tp_count % 2 == 0:
                        nc.scalar.copy(dst_ap, pst[:])
                    else:
                        nc.vector.tensor_copy(dst_ap, pst[:])
                    tp_count += 1

        transpose_blocks(sampleT, sample_bm)
        transpose_blocks(hiddenT, hidden_bm)
        transpose_blocks(mpT, mp_bm)

        def xhT_chunk(kc):
            return sampleT[:, kc, :] if kc < 4 else hiddenT[:, kc - 4, :]

        def xhdT_chunk(kc):
            if kc < 8:
                return xhT_chunk(kc)
            return mpT[:, kc - 8, :]

        def xrhT_chunk(kc):
            return sampleT[:, kc, :] if kc < 4 else rhT[:, kc - 4, :]

        def wload(w_ap, kc):
            wt = wpool.tile([P, 512], F32, tag="w")
            nc.sync.dma_start(wt[:], w_ap[kc * P : (kc + 1) * P, :])
            return wt

        # persistent elementwise state, [P, (bt, h)] layout
        d_sb = persist.tile([P, NT, 512], F32, tag="d")
        u_sb = persist.tile([P, NT, 512], F32, tag="u")
        ht_sb = persist.tile([P, NT, 512], F32, tag="ht")
        hn_sb = persist.tile([P, NT, 512], F32, tag="hn")
        s_sb = persist.tile([P, NT, 512], F32, tag="s")
        g_sb = persist.tile([P, NT, 512], F32, tag="g")

        def gemm_bm(x_chunk_fn, nk, w_ap, bias_idx, act_fn, out_tile):
            # batch-major: psum[bt] = X[bt,:] @ W  (+ bias row), then act
            ps = [gpsum.tile([P, 512], F32, name=f"gps{_}", tag="gps") for _ in range(NT)]
            for kc in range(nk):
                wt = wload(w_ap, kc)
                for bt in range(NT):
                    nc.tensor.matmul(
                        ps[bt][:],
                        x_chunk_fn(kc)[:, bt * P : (bt + 1) * P],
                        wt[:],
                        start=(kc == 0),
                        stop=(kc == nk - 1 and zero_bias),
                    )
            if not zero_bias:
                for bt in range(NT):
                    nc.tensor.matmul(
                        ps[bt][:],
                        ones[0:1, 0:P],
                        bias_sb[bias_idx : bias_idx + 1, :],
                        start=False,
                        stop=True,
                    )
            for bt in range(NT):
                nc.scalar.activation(out_tile[:, bt, :], ps[bt][:], act_fn)

        # ---- memory gate first (it gates the Horner chain) ----
        gemm_bm(xhdT_chunk, 12, Wm, 2, AF.Sigmoid, g_sb)
        nc.vector.tensor_scalar_mul(d_sb[:], g_sb[:], 0.5)
        nc.sync.dma_start(_r3(out_m), d_sb[:])

        # ---- reset gate, feature-major: rhT = sigmoid(.)^T * hiddenT ----
        psr = [gpsum.tile([P, 512], F32, name=f"gpsr{_}", tag="gps") for _ in range(NT)]
        for kc in range(8):
            wt = wload(Wr, kc)
            for oc in range(NT):
                nc.tensor.matmul(
                    psr[oc][:],
                    wt[:, oc * P : (oc + 1) * P],
                    xhT_chunk(kc)[:],
                    start=(kc == 0),
                    stop=(kc == 7 and zero_bias),
                )
        if not zero_bias:
            for oc in range(NT):
                nc.tensor.matmul(
                    psr[oc][:],
                    bias_sb[1:2, oc * P : (oc + 1) * P],
                    ones[0:1, :],
                    start=False,
                    stop=True,
                )
        for oc in range(NT):
            rt_s = scratch.tile([P, 512], F32, tag="rt")
            nc.scalar.activation(rt_s[:], psr[oc][:], AF.Sigmoid)
            nc.vector.tensor_tensor(rhT[:, oc, :], rt_s[:], hiddenT[:, oc, :], OP.mult)

        # ---- candidate and update gates ----
        gemm_bm(xrhT_chunk, 8, Wh, 3, AF.Tanh, ht_sb)
        gemm_bm(xhT_chunk, 8, Wu, 0, AF.Sigmoid, u_sb)

        # ---- Horner memory filter, streaming hiddens[k] forward ----
        h0 = None
        for k in range(K):
            if k < K - 1:
                hk = hpool.tile([P, NT, 512], F32, tag="hk")
                nc.sync.dma_start(hk[:], _r3(hiddens[k]))
            else:
                hk = hidden_bm  # hiddens[15] already resident
            if k >= 1:
                # updated_hiddens[k-1] = hiddens[k]; issue on the ACT HWDGE ring
                nc.scalar.dma_start(_r3(out_h[k - 1]), hk[:])
            if k == 0:
                h0 = hk
                continue
            i = K - k  # multiplier f_i = (i - d) / (i + 1)
            s_src = h0 if k == 1 else s_sb
            nc.vector.scalar_tensor_tensor(
                g_sb[:], d_sb[:], float(i), s_src[:], OP.subtract, OP.mult
            )
            nc.vector.scalar_tensor_tensor(
                s_sb[:], g_sb[:], -1.0 / (i + 1), hk[:], OP.mult, OP.add
            )

        # hn = d*s + u*ht
        nc.vector.tensor_tensor(g_sb[:], d_sb[:], s_sb[:], OP.mult)
        nc.vector.tensor_tensor(hn_sb[:], u_sb[:], ht_sb[:], OP.mult)
        nc.vector.tensor_tensor(hn_sb[:], hn_sb[:], g_sb[:], OP.add)
        nc.scalar.dma_start(_r3(out_h[K - 1]), hn_sb[:])

        # ---- output projection: y = hn @ Wo + bo ----
        transpose_blocks(hnT, hn_sb)
        psy = [gpsum.tile([P, 512], F32, name=f"gpsy{_}", tag="gps") for _ in range(NT)]
        for hc in range(4):
            wt = wload(Wo, hc)
            for bt in range(NT):
                nc.tensor.matmul(
                    psy[bt][:],
                    hnT[:, hc, bt * P : (bt + 1) * P],
                    wt[:],
                    start=(hc == 0),
                    stop=(hc == 3 and zero_bias),
                )
        if not zero_bias:
            for bt in range(NT):
                nc.tensor.matmul(
                    psy[bt][:], ones[0:1, 0:P], bias_sb[4:5, :], start=False, stop=True
                )
        for bt in range(NT):
            ys = scratch.tile([P, 512], F32, tag="ys")
            nc.scalar.copy(ys[:], psy[bt][:])
            nc.sync.dma_start(_r3(out_y)[:, bt, :], ys[:])

    nc.compile()
    return nc


_CACHE = {}


def _get_nc(zero_bias: bool):
    if zero_bias not in _CACHE:
        _CACHE[zero_bias] = _build(zero_bias)
    return _CACHE[zero_bias]


def _run(in_maps, zero_bias, **kwargs):
    nc = _get_nc(zero_bias)
    return bass_utils.run_bass_kernel_spmd(nc, in_maps, list(range(N_CORES)), **kwargs)


def make_in_maps(sample, hiddens, mem_para, Wu, bu, Wr, br, Wm, bm, Wh, bh, Wo, bo):
    a = lambda x: np.ascontiguousarray(np.asarray(x), dtype=np.float32)
    sample, hiddens, mem_para = a(sample), a(hiddens), a(mem_para)
    Wu, Wr, Wm, Wh, Wo = a(Wu), a(Wr), a(Wm), a(Wh), a(Wo)
    bu, br, bm, bh, bo = a(bu), a(br), a(bm), a(bh), a(bo)
    zero_bias = not any(np.any(b) for b in (bu, br, bm, bh, bo))
    in_maps = []
    for c in range(N_CORES):
        sl = slice(c * BL, (c + 1) * BL)
        m = {
            "sample": sample[sl],
            "hiddens": np.ascontiguousarray(hiddens[:, sl, :]),
            "mem_para": mem_para[sl],
            "Wu": Wu,
            "Wr": Wr,
            "Wm": Wm,
            "Wh": Wh,
            "Wo": Wo,
        }
        if not zero_bias:
            m.update({"bu": bu, "br": br, "bm": bm, "bh": bh, "bo": bo})
        in_maps.append(m)
    return in_maps, zero_bias


def assemble(results):
    output = np.concatenate([results[c]["out_y"] for c in range(N_CORES)], axis=0)
    updated = np.concatenate([results[c]["out_h"] for c in range(N_CORES)], axis=1)
    mgate = np.concatenate([results[c]["out_m"] for c in range(N_CORES)], axis=0)
    return output, updated, mgate


def kernel(**inputs):
    in_maps, zero_bias = make_in_maps(**inputs)
    res = _run(in_maps, zero_bias)
    return assemble(res.results)
